# revision 56
# baseline (speedup 1.0000x reference)
"""LinOSS layer Trainium2 kernel, v6.

Math (rank-2 trig decomposition): the per-state recurrence matrix
M = [[1, -sA], [s, 1-s^2 A]] has eigenvalues e^{+-i theta},
cos(theta) = 1 - s^2 A / 2.  The scanned state collapses to

    u_t = s * Bu_t
    E   = cumsum(T1 * u);  F = cumsum(T2 * u)
    T1  = gamma*cos(t th) + sin(t th);  T2 = cos(t th) - gamma*sin(t th)
    x_t = sin(t th) * E_t + cos(t th) * F_t
    gamma = (s - s^2 A / 2) / sin(theta)

Host precomputes: the transposed input (fold-interleaved columns), all
tables (fp64 -> bf16, laid out in per-quarter blocks for just-in-time
DMA), fold-1 scan initials, and the final input*D + 8-way partial sum
(plus un-interleaving the quarter-major device output) in the gather.

Measured constraints driving the structure: the DVE scan runs at
~2.2 ns/col (2 arrays x 4096 cols is the serial floor), DVE bf16
tensor-tensor ops run ~0.65 ns/col with a ~0.2 us fixed cost (so ops
are 1024 wide), and DMA delivers ~190 GB/s aggregate with ~1 us issue
cost per instruction (so transfers are consolidated and streamed
just-in-time: the first input block rides in the prefix tensor and is
consumed straight from it).  One fused loop per 1024-col quarter:
Bu matmuls -> u evac -> modulate -> chained scans -> demod -> psum-
accumulated projection -> evac into a per-quarter staging tile ->
one output DMA per quarter (quarter-major DRAM layout).
"""

import numpy as np

L, H, P = 8192, 128, 256
NCORES = 8
SLOC = P // NCORES          # 32 states per core
FOLD = 2
CL = L // FOLD              # 4096 free columns
JT = 512
NJT = CL // JT              # 8 chunks
HALF = 2 * SLOC             # 64 = (ri, s) rows per fold
SQ = 1024                   # quarter width
NSQ = CL // SQ              # 4 quarters
PRE = HALF + H + 2          # Bt|Cpk|init cols in the prefix tensor
PIN = 2 * SQ                # input cols riding in the prefix tensor

_CACHE: dict = {}


def _build_bass(split_waits=True):
    import concourse.bass as bass
    import concourse.mybir as mybir
    import concourse.tile as tile

    dt = mybir.dt.float32
    bt = mybir.dt.bfloat16
    Alu = mybir.AluOpType

    nc = bass.Bass(
        trn_type="TRN2",
        target_bir_lowering=False,
        debug=False,
        num_devices=NCORES,
    )

    # pre: Bt|Cpk|init(bf16)|inpT cols 0:1024 (input is fold-interleaved:
    # chunk jt fold c at cols jt*1024 + c*512)
    pre_d = nc.dram_tensor("pre", [128, PRE + PIN], bt, kind="ExternalInput").ap()
    inpR_d = nc.dram_tensor("inpR", [H, L - PIN], bt, kind="ExternalInput").ap()
    # TAB1: [T1q|T2q] per-quarter blocks of 1024 -> (128, 2048) per q
    TAB1_d = nc.dram_tensor("TAB1", [128, 2 * CL], bt, kind="ExternalInput").ap()
    # TAB2: [Snq|Csq] per-quarter blocks
    TAB2_d = nc.dram_tensor("TAB2", [128, 2 * CL], bt, kind="ExternalInput").ap()
    # quarter-major output: [q0f0|q0f1|q1f0|q1f1|...]
    outp = nc.dram_tensor("outp", [H, L], bt, kind="ExternalOutput").ap()

    with tile.TileContext(nc) as tc:
        cpool = tc.alloc_tile_pool(name="const", bufs=1)
        big = tc.alloc_tile_pool(name="big", bufs=1)
        work = tc.alloc_tile_pool(name="work", bufs=3)
        evac = tc.alloc_tile_pool(name="evac", bufs=3)
        psum_bu = tc.alloc_tile_pool(name="psum_bu", bufs=2, space="PSUM")
        psum_o = tc.alloc_tile_pool(name="psum_o", bufs=2, space="PSUM")

        pre = cpool.tile([128, PRE + PIN], bt)
        Bt = pre[:, 0:HALF]
        Cpk = pre[:, HALF : HALF + H]
        initb = pre[:, HALF + H : HALF + H + 2]
        inpT = big.tile([H, L], bt, tag="inpT")   # cols 0:1024 unused
        TAB1 = big.tile([128, 2 * CL], bt, tag="TAB1")
        TAB2 = big.tile([128, 2 * CL], bt, tag="TAB2")

        def T1q(q):
            return TAB1[:, q * 2 * SQ : q * 2 * SQ + SQ]

        def T2q(q):
            return TAB1[:, q * 2 * SQ + SQ : (q + 1) * 2 * SQ]

        def Snq(q):
            return TAB2[:, q * 2 * SQ : q * 2 * SQ + SQ]

        def Csq(q):
            return TAB2[:, q * 2 * SQ + SQ : (q + 1) * 2 * SQ]

        # qSP: prefix (incl. first 2 input chunks), then just-in-time
        # interleave of input quarters with demod-table quarters
        nc.sync.dma_start(out=pre[:], in_=pre_d)
        nc.sync.dma_start(out=inpT[:, 2 * SQ : 4 * SQ], in_=inpR_d[:, 0 : 2 * SQ])
        nc.sync.dma_start(out=TAB2[:, 0 : 2 * SQ], in_=TAB2_d[:, 0 : 2 * SQ])
        nc.sync.dma_start(out=inpT[:, 4 * SQ : 6 * SQ], in_=inpR_d[:, 2 * SQ : 4 * SQ])
        nc.sync.dma_start(out=TAB2[:, 2 * SQ : 4 * SQ], in_=TAB2_d[:, 2 * SQ : 4 * SQ])
        nc.sync.dma_start(out=inpT[:, 6 * SQ : 8 * SQ], in_=inpR_d[:, 4 * SQ :])
        nc.sync.dma_start(out=TAB2[:, 4 * SQ : 6 * SQ], in_=TAB2_d[:, 4 * SQ : 6 * SQ])
        nc.sync.dma_start(out=TAB2[:, 6 * SQ :], in_=TAB2_d[:, 6 * SQ :])
        # qACT: mod tables per-quarter-pair (just-in-time)
        nc.scalar.dma_start(out=TAB1[:, 0 : 2 * SQ], in_=TAB1_d[:, 0 : 2 * SQ])
        nc.scalar.dma_start(out=TAB1[:, 2 * SQ : 4 * SQ], in_=TAB1_d[:, 2 * SQ : 4 * SQ])
        nc.scalar.dma_start(out=TAB1[:, 4 * SQ : 8 * SQ], in_=TAB1_d[:, 4 * SQ :])

        # prewarm the ACT function table while DMAs stream
        warm = cpool.tile([128, 2], bt)
        nc.scalar.copy(warm[:], pre[:, 0:2])

        ones = cpool.tile([128, SQ], bt)
        nc.gpsimd.memset(ones[:], 1.0)

        Y1 = big.tile([128, CL], bt, tag="Y1")
        Y2 = big.tile([128, CL], bt, tag="Y2")
        E = big.tile([128, CL], bt, tag="E")
        F = big.tile([128, CL], bt, tag="F")

        fh = slice(0, HALF)
        sh = slice(HALF, 128)

        for q in range(NSQ):
            qs = slice(q * SQ, (q + 1) * SQ)
            # Bu for the quarter's two 512-col chunks into one 2-bank psum
            pbu = psum_bu.tile([128, SQ], dt, tag="bu")
            for h in range(2):
                jt = 2 * q + h
                for c in range(FOLD):
                    col = jt * 2 * JT + c * JT
                    rhs = (
                        pre[:, PRE + col : PRE + col + JT]
                        if col < PIN
                        else inpT[:, col : col + JT]
                    )
                    nc.tensor.matmul(
                        pbu[c * HALF : (c + 1) * HALF, h * JT : (h + 1) * JT],
                        Bt, rhs,
                        start=True, stop=True, tile_position=(0, c * HALF),
                    )
            u = evac.tile([128, SQ], bt, tag="u")
            nc.scalar.copy(u[:], pbu[:])
            nc.vector.tensor_mul(Y1[:, qs], u[:], T1q(q))
            nc.vector.tensor_mul(Y2[:, qs], u[:], T2q(q))
            # chained scans
            iE = initb[:, 0:1] if q == 0 else E[:, q * SQ - 1 : q * SQ]
            iF = initb[:, 1:2] if q == 0 else F[:, q * SQ - 1 : q * SQ]
            bass.BassGpSimd.tensor_tensor_scan(
                nc.vector, E[:, qs], ones[:], Y1[:, qs], iE, Alu.mult, Alu.add
            )
            bass.BassGpSimd.tensor_tensor_scan(
                nc.vector, F[:, qs], ones[:], Y2[:, qs], iF, Alu.mult, Alu.add
            )
            # demod (full quarter) + projection with add folded into PSUM
            m1 = work.tile([128, SQ], bt, tag="m1")
            m2 = work.tile([128, SQ], bt, tag="m2")
            nc.vector.tensor_mul(m1[:], E[:, qs], Snq(q))
            nc.vector.tensor_mul(m2[:], F[:, qs], Csq(q))
            osb = evac.tile([128, 2 * SQ], bt, tag="osb")
            if q == NSQ - 1:
                # tail: explicit x-add on the now-idle DVE makes each
                # fold's projection a single-matmul group, unblocking
                # the evac chain earlier
                x = work.tile([128, SQ], bt, tag="x")
                nc.vector.tensor_add(x[:], m1[:], m2[:])
            for c in range(FOLD):
                ps = fh if c == 0 else sh
                po = psum_o.tile([128, SQ], dt, tag="out")
                for h in range(2):
                    hs = slice(h * JT, (h + 1) * JT)
                    if q == NSQ - 1:
                        nc.tensor.matmul(
                            po[:, hs], Cpk[ps, :], x[ps, hs], start=True,
                            stop=True, tile_position=(c * HALF, 0),
                        )
                    else:
                        nc.tensor.matmul(
                            po[:, hs], Cpk[ps, :], m1[ps, hs], start=True,
                            stop=False, tile_position=(c * HALF, 0),
                        )
                        nc.tensor.matmul(
                            po[:, hs], Cpk[ps, :], m2[ps, hs], start=False,
                            stop=True, tile_position=(c * HALF, 0),
                        )
                nc.scalar.copy(osb[:, c * SQ : (c + 1) * SQ], po[:])
            if q < NSQ - 1:
                eng = nc.scalar if q % 2 == 0 else nc.sync
                eng.dma_start(
                    out=outp[:, q * 2 * SQ : (q + 1) * 2 * SQ], in_=osb[:]
                )
            else:
                # last quarter: split halves across both queues to
                # halve the final drain
                nc.sync.dma_start(
                    out=outp[:, q * 2 * SQ : q * 2 * SQ + SQ],
                    in_=osb[:, 0:SQ],
                )
                nc.scalar.dma_start(
                    out=outp[:, q * 2 * SQ + SQ : (q + 1) * 2 * SQ],
                    in_=osb[:, SQ:],
                )

        for p in (psum_o, psum_bu, evac, work, big, cpool):
            p.release()
    if split_waits:
        _split_matmul_waits(nc, mybir)
    return nc


def _split_matmul_waits(nc, mybir):
    """Hardware instruction structs fit a limited number of embedded sync
    waits; move extra waits onto an inserted same-queue no-op."""
    caps = {"InstMatmult": 1}
    skip = {"InstNoOp", "InstAllEngineBarrier", "InstSync"}
    k = 0
    for bb in nc.main_func.blocks:
        insts = bb.instructions
        i = 0
        while i < len(insts):
            ins = insts[i]
            tn = type(ins).__name__
            if tn not in skip and ins.sync_info is not None:
                cap = caps.get(tn, 1)
                w = list(ins.sync_info.on_wait or [])
                if len(w) > cap:
                    for wj in w[:-cap]:
                        nop = mybir.InstNoOp(
                            name=f"I-mmdep-{k}",
                            engine=ins.engine,
                            ins=[],
                            outs=[],
                            sync_info=mybir.SyncInfo(
                                on_wait=[wj], on_update=[]
                            ),
                        )
                        k += 1
                        insts.insert(i, nop)
                        i += 1
                    ins.sync_info = mybir.SyncInfo(
                        on_wait=w[-cap:], on_update=ins.sync_info.on_update
                    )
            i += 1


def _host_prep(inputs):
    import ml_dtypes

    bf16 = ml_dtypes.bfloat16

    inp64 = np.asarray(inputs["input_sequence"], np.float64)
    inpT_n = inp64.T.astype(bf16)                  # (H, L) natural
    # interleave fold-0/fold-1 512-col chunks: [c0f0|c0f1|c1f0|c1f1...]
    inpT = np.empty((H, L), bf16)
    for jt in range(NJT):
        inpT[:, jt * 2 * JT : jt * 2 * JT + JT] = \
            inpT_n[:, jt * JT : (jt + 1) * JT]
        inpT[:, jt * 2 * JT + JT : (jt + 1) * 2 * JT] = \
            inpT_n[:, CL + jt * JT : CL + (jt + 1) * JT]

    A = np.maximum(np.asarray(inputs["A_diag_raw"], np.float64), 0.0)
    s = 1.0 / (1.0 + np.exp(-np.asarray(inputs["steps_raw"], np.float64)))
    Br = np.asarray(inputs["B_real"], np.float64)
    Bi = np.asarray(inputs["B_img"], np.float64)
    Cr = np.asarray(inputs["C_real"], np.float64)
    Ci = np.asarray(inputs["C_img"], np.float64)

    costh = 1.0 - s * s * A / 2.0
    sinth = np.sqrt(np.maximum(1.0 - costh * costh, 1e-300))
    theta = np.arctan2(sinth, costh)
    gamma = (s - s * s * A / 2.0) / sinth

    # fold-1 scan initials: E/F totals over the fold-0 half (fp64)
    sBr = s[:, None] * Br          # (P, H)
    sBi = s[:, None] * Bi
    u_r0 = inp64[:CL] @ sBr.T      # (CL, P)
    u_i0 = inp64[:CL] @ sBi.T
    t0 = np.arange(CL, dtype=np.float64)
    ang0 = t0[:, None] * theta[None, :]
    sn0, cs0 = np.sin(ang0), np.cos(ang0)
    t1_0 = gamma[None, :] * cs0 + sn0
    t2_0 = cs0 - gamma[None, :] * sn0
    E0_r = (t1_0 * u_r0).sum(axis=0)
    E0_i = (t1_0 * u_i0).sum(axis=0)
    F0_r = (t2_0 * u_r0).sum(axis=0)
    F0_i = (t2_0 * u_i0).sum(axis=0)

    twopi = 2.0 * np.pi
    t_in = np.arange(CL, dtype=np.float64)
    in_maps = []
    for k in range(NCORES):
        sl = slice(k * SLOC, (k + 1) * SLOC)
        th = theta[sl]
        gm = gamma[sl]

        pre = np.empty((128, PRE + PIN), bf16)
        pre[:, 0:SLOC] = sBr[sl].T.astype(bf16)
        pre[:, SLOC:HALF] = sBi[sl].T.astype(bf16)
        for c in range(FOLD):
            pre[c * HALF : c * HALF + SLOC, HALF : HALF + H] = \
                Cr[:, sl].T.astype(bf16)
            pre[c * HALF + SLOC : (c + 1) * HALF, HALF : HALF + H] = \
                (-Ci[:, sl].T).astype(bf16)
        init = np.zeros((128, 2), np.float64)
        init[HALF : HALF + SLOC, 0] = E0_r[sl]
        init[HALF + SLOC :, 0] = E0_i[sl]
        init[HALF : HALF + SLOC, 1] = F0_r[sl]
        init[HALF + SLOC :, 1] = F0_i[sl]
        pre[:, HALF + H : HALF + H + 2] = init.astype(bf16)
        pre[:, PRE : PRE + PIN] = inpT[:, 0:PIN]

        # per-quarter table blocks: TAB1 = [T1q|T2q]*4, TAB2 = [Snq|Csq]*4
        TAB1 = np.empty((128, 2 * CL), bf16)
        TAB2 = np.empty((128, 2 * CL), bf16)
        sn_f = np.empty((128, CL), np.float64)
        cs_f = np.empty((128, CL), np.float64)
        t1_f = np.empty((128, CL), np.float64)
        t2_f = np.empty((128, CL), np.float64)
        for c in range(FOLD):
            ang = np.mod((c * CL + t_in)[None, :] * th[:, None], twopi)
            sn = np.sin(ang)
            cs = np.cos(ang)
            t1 = gm[:, None] * cs + sn
            t2 = cs - gm[:, None] * sn
            for ri in range(2):
                rs = slice(c * HALF + ri * SLOC, c * HALF + (ri + 1) * SLOC)
                sn_f[rs] = sn
                cs_f[rs] = cs
                t1_f[rs] = t1
                t2_f[rs] = t2
        for q in range(NSQ):
            qs = slice(q * SQ, (q + 1) * SQ)
            TAB1[:, q * 2 * SQ : q * 2 * SQ + SQ] = t1_f[:, qs].astype(bf16)
            TAB1[:, q * 2 * SQ + SQ : (q + 1) * 2 * SQ] = \
                t2_f[:, qs].astype(bf16)
            TAB2[:, q * 2 * SQ : q * 2 * SQ + SQ] = sn_f[:, qs].astype(bf16)
            TAB2[:, q * 2 * SQ + SQ : (q + 1) * 2 * SQ] = \
                cs_f[:, qs].astype(bf16)

        in_maps.append({
            "pre": pre,
            "inpR": np.ascontiguousarray(inpT[:, PIN:]),
            "TAB1": TAB1,
            "TAB2": TAB2,
        })
    return in_maps


LAST_RESULTS = None


def kernel(**inputs) -> np.ndarray:
    global LAST_RESULTS
    from concourse.bass_utils import run_bass_kernel_spmd

    if "nc" not in _CACHE:
        _CACHE["nc"] = _build_bass()
    nc = _CACHE["nc"]

    in_maps = _host_prep(inputs)
    res = run_bass_kernel_spmd(nc, in_maps, core_ids=list(range(NCORES)))
    LAST_RESULTS = res
    part = np.zeros((H, L), np.float32)
    for r in res.results:
        part += np.asarray(r["outp"], np.float32)
    # un-interleave the quarter-major layout: [q0f0|q0f1|q1f0|q1f1|...]
    y = np.empty((H, L), np.float32)
    for q in range(NSQ):
        y[:, q * SQ : (q + 1) * SQ] = part[:, q * 2 * SQ : q * 2 * SQ + SQ]
        y[:, CL + q * SQ : CL + (q + 1) * SQ] = \
            part[:, q * 2 * SQ + SQ : (q + 1) * 2 * SQ]
    out = y.T + np.asarray(inputs["input_sequence"], np.float32) * np.asarray(
        inputs["D"], np.float32
    )
    return np.ascontiguousarray(out)


# revision 57
# speedup vs baseline: 1.0601x; 1.0601x over previous
"""LinOSS layer Trainium2 kernel, v6.

Math (rank-2 trig decomposition): the per-state recurrence matrix
M = [[1, -sA], [s, 1-s^2 A]] has eigenvalues e^{+-i theta},
cos(theta) = 1 - s^2 A / 2.  The scanned state collapses to

    u_t = s * Bu_t
    E   = cumsum(T1 * u);  F = cumsum(T2 * u)
    T1  = gamma*cos(t th) + sin(t th);  T2 = cos(t th) - gamma*sin(t th)
    x_t = sin(t th) * E_t + cos(t th) * F_t
    gamma = (s - s^2 A / 2) / sin(theta)

Host precomputes: the transposed input (fold-interleaved columns), all
tables (fp64 -> bf16, laid out in per-quarter blocks for just-in-time
DMA), fold-1 scan initials, and the final input*D + 8-way partial sum
(plus un-interleaving the quarter-major device output) in the gather.

Measured constraints driving the structure: the DVE scan runs at
~2.2 ns/col (2 arrays x 4096 cols is the serial floor), DVE bf16
tensor-tensor ops run ~0.65 ns/col with a ~0.2 us fixed cost (so ops
are 1024 wide), and DMA delivers ~190 GB/s aggregate with ~1 us issue
cost per instruction (so transfers are consolidated and streamed
just-in-time: the first input block rides in the prefix tensor and is
consumed straight from it).  One fused loop per 1024-col quarter:
Bu matmuls -> u evac -> modulate -> chained scans -> demod -> psum-
accumulated projection -> evac into a per-quarter staging tile ->
one output DMA per quarter (quarter-major DRAM layout).
"""

import numpy as np

L, H, P = 8192, 128, 256
NCORES = 8
SLOC = P // NCORES          # 32 states per core
FOLD = 2
CL = L // FOLD              # 4096 free columns
JT = 512
NJT = CL // JT              # 8 chunks
HALF = 2 * SLOC             # 64 = (ri, s) rows per fold
SQ = 1024                   # quarter width
NSQ = CL // SQ              # 4 quarters
PRE = HALF + H + 2          # Bt|Cpk|init cols in the prefix tensor
PIN = 2 * SQ                # input cols riding in the prefix tensor

_CACHE: dict = {}


def _build_bass(split_waits=True):
    import concourse.bass as bass
    import concourse.mybir as mybir
    import concourse.tile as tile

    dt = mybir.dt.float32
    bt = mybir.dt.bfloat16
    Alu = mybir.AluOpType

    nc = bass.Bass(
        trn_type="TRN2",
        target_bir_lowering=False,
        debug=False,
        num_devices=NCORES,
    )

    # pre: Bt|Cpk|init(bf16)|inpT cols 0:1024 (input is fold-interleaved:
    # chunk jt fold c at cols jt*1024 + c*512)
    pre_d = nc.dram_tensor("pre", [128, PRE + PIN], bt, kind="ExternalInput").ap()
    inpR_d = nc.dram_tensor("inpR", [H, L - PIN], bt, kind="ExternalInput").ap()
    # TAB1: [T1q|T2q] per-quarter blocks of 1024 -> (128, 2048) per q
    TAB1_d = nc.dram_tensor("TAB1", [128, 2 * CL], bt, kind="ExternalInput").ap()
    # TAB2: [Snq|Csq] per-quarter blocks
    TAB2_d = nc.dram_tensor("TAB2", [128, 2 * CL], bt, kind="ExternalInput").ap()
    # quarter-major output: [q0f0|q0f1|q1f0|q1f1|...]
    outp = nc.dram_tensor("outp", [H, L], bt, kind="ExternalOutput").ap()

    with tile.TileContext(nc) as tc:
        cpool = tc.alloc_tile_pool(name="const", bufs=1)
        big = tc.alloc_tile_pool(name="big", bufs=1)
        work = tc.alloc_tile_pool(name="work", bufs=3)
        evac = tc.alloc_tile_pool(name="evac", bufs=3)
        psum_bu = tc.alloc_tile_pool(name="psum_bu", bufs=2, space="PSUM")
        psum_o = tc.alloc_tile_pool(name="psum_o", bufs=2, space="PSUM")

        pre = cpool.tile([128, PRE + PIN], bt)
        Bt = pre[:, 0:HALF]
        Cpk = pre[:, HALF : HALF + H]
        initb = pre[:, HALF + H : HALF + H + 2]
        inpT = big.tile([H, L], bt, tag="inpT")   # cols 0:1024 unused
        TAB1 = big.tile([128, 2 * CL], bt, tag="TAB1")
        TAB2 = big.tile([128, 2 * CL], bt, tag="TAB2")

        def T1q(q):
            return TAB1[:, q * 2 * SQ : q * 2 * SQ + SQ]

        def T2q(q):
            return TAB1[:, q * 2 * SQ + SQ : (q + 1) * 2 * SQ]

        def Snq(q):
            return TAB2[:, q * 2 * SQ : q * 2 * SQ + SQ]

        def Csq(q):
            return TAB2[:, q * 2 * SQ + SQ : (q + 1) * 2 * SQ]

        # qSP: prefix (incl. first 2 input chunks), then just-in-time
        # interleave of input quarters with demod-table quarters
        nc.sync.dma_start(out=pre[:], in_=pre_d)
        nc.sync.dma_start(out=inpT[:, 2 * SQ : 4 * SQ], in_=inpR_d[:, 0 : 2 * SQ])
        nc.sync.dma_start(out=TAB2[:, 0 : 2 * SQ], in_=TAB2_d[:, 0 : 2 * SQ])
        nc.sync.dma_start(out=inpT[:, 4 * SQ : 6 * SQ], in_=inpR_d[:, 2 * SQ : 4 * SQ])
        nc.sync.dma_start(out=TAB2[:, 2 * SQ : 4 * SQ], in_=TAB2_d[:, 2 * SQ : 4 * SQ])
        nc.sync.dma_start(out=inpT[:, 6 * SQ : 8 * SQ], in_=inpR_d[:, 4 * SQ :])
        nc.sync.dma_start(out=TAB2[:, 4 * SQ : 6 * SQ], in_=TAB2_d[:, 4 * SQ : 6 * SQ])
        nc.sync.dma_start(out=TAB2[:, 6 * SQ :], in_=TAB2_d[:, 6 * SQ :])
        # qACT: mod tables per-quarter-pair (just-in-time)
        nc.scalar.dma_start(out=TAB1[:, 0 : 2 * SQ], in_=TAB1_d[:, 0 : 2 * SQ])
        nc.scalar.dma_start(out=TAB1[:, 2 * SQ : 4 * SQ], in_=TAB1_d[:, 2 * SQ : 4 * SQ])
        nc.scalar.dma_start(out=TAB1[:, 4 * SQ : 8 * SQ], in_=TAB1_d[:, 4 * SQ :])

        # prewarm the ACT function table while DMAs stream
        warm = cpool.tile([128, 2], bt)
        nc.scalar.copy(warm[:], pre[:, 0:2])

        ones = cpool.tile([128, SQ], bt)
        nc.gpsimd.memset(ones[:], 1.0)

        Y1 = big.tile([128, CL], bt, tag="Y1")
        Y2 = big.tile([128, CL], bt, tag="Y2")
        E = big.tile([128, CL], bt, tag="E")
        F = big.tile([128, CL], bt, tag="F")

        fh = slice(0, HALF)
        sh = slice(HALF, 128)

        for q in range(NSQ):
            qs = slice(q * SQ, (q + 1) * SQ)
            # Bu for the quarter's two 512-col chunks into one 2-bank psum
            pbu = psum_bu.tile([128, SQ], dt, tag="bu")
            for h in range(2):
                jt = 2 * q + h
                for c in range(FOLD):
                    col = jt * 2 * JT + c * JT
                    rhs = (
                        pre[:, PRE + col : PRE + col + JT]
                        if col < PIN
                        else inpT[:, col : col + JT]
                    )
                    nc.tensor.matmul(
                        pbu[c * HALF : (c + 1) * HALF, h * JT : (h + 1) * JT],
                        Bt, rhs,
                        start=True, stop=True, tile_position=(0, c * HALF),
                    )
            u = evac.tile([128, SQ], bt, tag="u")
            nc.scalar.copy(u[:], pbu[:])
            nc.vector.tensor_mul(Y1[:, qs], u[:], T1q(q))
            nc.vector.tensor_mul(Y2[:, qs], u[:], T2q(q))
            # chained scans
            iE = initb[:, 0:1] if q == 0 else E[:, q * SQ - 1 : q * SQ]
            iF = initb[:, 1:2] if q == 0 else F[:, q * SQ - 1 : q * SQ]
            bass.BassGpSimd.tensor_tensor_scan(
                nc.vector, E[:, qs], ones[:], Y1[:, qs], iE, Alu.mult, Alu.add
            )
            bass.BassGpSimd.tensor_tensor_scan(
                nc.vector, F[:, qs], ones[:], Y2[:, qs], iF, Alu.mult, Alu.add
            )
            # demod (full quarter) + projection with add folded into PSUM
            m1 = work.tile([128, SQ], bt, tag="m1")
            m2 = work.tile([128, SQ], bt, tag="m2")
            nc.vector.tensor_mul(m1[:], E[:, qs], Snq(q))
            nc.vector.tensor_mul(m2[:], F[:, qs], Csq(q))
            osb = evac.tile([128, 2 * SQ], bt, tag="osb")
            for c in range(FOLD):
                ps = fh if c == 0 else sh
                po = psum_o.tile([128, SQ], dt, tag="out")
                for h in range(2):
                    hs = slice(h * JT, (h + 1) * JT)
                    nc.tensor.matmul(
                        po[:, hs], Cpk[ps, :], m1[ps, hs], start=True,
                        stop=False, tile_position=(c * HALF, 0),
                    )
                    nc.tensor.matmul(
                        po[:, hs], Cpk[ps, :], m2[ps, hs], start=False,
                        stop=True, tile_position=(c * HALF, 0),
                    )
                nc.scalar.copy(osb[:, c * SQ : (c + 1) * SQ], po[:])
            if q < NSQ - 1:
                eng = nc.scalar if q % 2 == 0 else nc.sync
                eng.dma_start(
                    out=outp[:, q * 2 * SQ : (q + 1) * 2 * SQ], in_=osb[:]
                )
            else:
                # last quarter: split halves across both queues to
                # halve the final drain
                nc.sync.dma_start(
                    out=outp[:, q * 2 * SQ : q * 2 * SQ + SQ],
                    in_=osb[:, 0:SQ],
                )
                nc.scalar.dma_start(
                    out=outp[:, q * 2 * SQ + SQ : (q + 1) * 2 * SQ],
                    in_=osb[:, SQ:],
                )

        for p in (psum_o, psum_bu, evac, work, big, cpool):
            p.release()
    if split_waits:
        _split_matmul_waits(nc, mybir)
    return nc


def _split_matmul_waits(nc, mybir):
    """Hardware instruction structs fit a limited number of embedded sync
    waits; move extra waits onto an inserted same-queue no-op."""
    caps = {"InstMatmult": 1}
    skip = {"InstNoOp", "InstAllEngineBarrier", "InstSync"}
    k = 0
    for bb in nc.main_func.blocks:
        insts = bb.instructions
        i = 0
        while i < len(insts):
            ins = insts[i]
            tn = type(ins).__name__
            if tn not in skip and ins.sync_info is not None:
                cap = caps.get(tn, 1)
                w = list(ins.sync_info.on_wait or [])
                if len(w) > cap:
                    for wj in w[:-cap]:
                        nop = mybir.InstNoOp(
                            name=f"I-mmdep-{k}",
                            engine=ins.engine,
                            ins=[],
                            outs=[],
                            sync_info=mybir.SyncInfo(
                                on_wait=[wj], on_update=[]
                            ),
                        )
                        k += 1
                        insts.insert(i, nop)
                        i += 1
                    ins.sync_info = mybir.SyncInfo(
                        on_wait=w[-cap:], on_update=ins.sync_info.on_update
                    )
            i += 1


def _host_prep(inputs):
    import ml_dtypes

    bf16 = ml_dtypes.bfloat16

    inp64 = np.asarray(inputs["input_sequence"], np.float64)
    inpT_n = inp64.T.astype(bf16)                  # (H, L) natural
    # interleave fold-0/fold-1 512-col chunks: [c0f0|c0f1|c1f0|c1f1...]
    inpT = np.empty((H, L), bf16)
    for jt in range(NJT):
        inpT[:, jt * 2 * JT : jt * 2 * JT + JT] = \
            inpT_n[:, jt * JT : (jt + 1) * JT]
        inpT[:, jt * 2 * JT + JT : (jt + 1) * 2 * JT] = \
            inpT_n[:, CL + jt * JT : CL + (jt + 1) * JT]

    A = np.maximum(np.asarray(inputs["A_diag_raw"], np.float64), 0.0)
    s = 1.0 / (1.0 + np.exp(-np.asarray(inputs["steps_raw"], np.float64)))
    Br = np.asarray(inputs["B_real"], np.float64)
    Bi = np.asarray(inputs["B_img"], np.float64)
    Cr = np.asarray(inputs["C_real"], np.float64)
    Ci = np.asarray(inputs["C_img"], np.float64)

    costh = 1.0 - s * s * A / 2.0
    sinth = np.sqrt(np.maximum(1.0 - costh * costh, 1e-300))
    theta = np.arctan2(sinth, costh)
    gamma = (s - s * s * A / 2.0) / sinth

    # fold-1 scan initials: E/F totals over the fold-0 half (fp64)
    sBr = s[:, None] * Br          # (P, H)
    sBi = s[:, None] * Bi
    u_r0 = inp64[:CL] @ sBr.T      # (CL, P)
    u_i0 = inp64[:CL] @ sBi.T
    t0 = np.arange(CL, dtype=np.float64)
    ang0 = t0[:, None] * theta[None, :]
    sn0, cs0 = np.sin(ang0), np.cos(ang0)
    t1_0 = gamma[None, :] * cs0 + sn0
    t2_0 = cs0 - gamma[None, :] * sn0
    E0_r = (t1_0 * u_r0).sum(axis=0)
    E0_i = (t1_0 * u_i0).sum(axis=0)
    F0_r = (t2_0 * u_r0).sum(axis=0)
    F0_i = (t2_0 * u_i0).sum(axis=0)

    twopi = 2.0 * np.pi
    t_in = np.arange(CL, dtype=np.float64)
    in_maps = []
    for k in range(NCORES):
        sl = slice(k * SLOC, (k + 1) * SLOC)
        th = theta[sl]
        gm = gamma[sl]

        pre = np.empty((128, PRE + PIN), bf16)
        pre[:, 0:SLOC] = sBr[sl].T.astype(bf16)
        pre[:, SLOC:HALF] = sBi[sl].T.astype(bf16)
        for c in range(FOLD):
            pre[c * HALF : c * HALF + SLOC, HALF : HALF + H] = \
                Cr[:, sl].T.astype(bf16)
            pre[c * HALF + SLOC : (c + 1) * HALF, HALF : HALF + H] = \
                (-Ci[:, sl].T).astype(bf16)
        init = np.zeros((128, 2), np.float64)
        init[HALF : HALF + SLOC, 0] = E0_r[sl]
        init[HALF + SLOC :, 0] = E0_i[sl]
        init[HALF : HALF + SLOC, 1] = F0_r[sl]
        init[HALF + SLOC :, 1] = F0_i[sl]
        pre[:, HALF + H : HALF + H + 2] = init.astype(bf16)
        pre[:, PRE : PRE + PIN] = inpT[:, 0:PIN]

        # per-quarter table blocks: TAB1 = [T1q|T2q]*4, TAB2 = [Snq|Csq]*4
        TAB1 = np.empty((128, 2 * CL), bf16)
        TAB2 = np.empty((128, 2 * CL), bf16)
        sn_f = np.empty((128, CL), np.float64)
        cs_f = np.empty((128, CL), np.float64)
        t1_f = np.empty((128, CL), np.float64)
        t2_f = np.empty((128, CL), np.float64)
        for c in range(FOLD):
            ang = np.mod((c * CL + t_in)[None, :] * th[:, None], twopi)
            sn = np.sin(ang)
            cs = np.cos(ang)
            t1 = gm[:, None] * cs + sn
            t2 = cs - gm[:, None] * sn
            for ri in range(2):
                rs = slice(c * HALF + ri * SLOC, c * HALF + (ri + 1) * SLOC)
                sn_f[rs] = sn
                cs_f[rs] = cs
                t1_f[rs] = t1
                t2_f[rs] = t2
        for q in range(NSQ):
            qs = slice(q * SQ, (q + 1) * SQ)
            TAB1[:, q * 2 * SQ : q * 2 * SQ + SQ] = t1_f[:, qs].astype(bf16)
            TAB1[:, q * 2 * SQ + SQ : (q + 1) * 2 * SQ] = \
                t2_f[:, qs].astype(bf16)
            TAB2[:, q * 2 * SQ : q * 2 * SQ + SQ] = sn_f[:, qs].astype(bf16)
            TAB2[:, q * 2 * SQ + SQ : (q + 1) * 2 * SQ] = \
                cs_f[:, qs].astype(bf16)

        in_maps.append({
            "pre": pre,
            "inpR": np.ascontiguousarray(inpT[:, PIN:]),
            "TAB1": TAB1,
            "TAB2": TAB2,
        })
    return in_maps


LAST_RESULTS = None


def kernel(**inputs) -> np.ndarray:
    global LAST_RESULTS
    from concourse.bass_utils import run_bass_kernel_spmd

    if "nc" not in _CACHE:
        _CACHE["nc"] = _build_bass()
    nc = _CACHE["nc"]

    in_maps = _host_prep(inputs)
    res = run_bass_kernel_spmd(nc, in_maps, core_ids=list(range(NCORES)))
    LAST_RESULTS = res
    part = np.zeros((H, L), np.float32)
    for r in res.results:
        part += np.asarray(r["outp"], np.float32)
    # un-interleave the quarter-major layout: [q0f0|q0f1|q1f0|q1f1|...]
    y = np.empty((H, L), np.float32)
    for q in range(NSQ):
        y[:, q * SQ : (q + 1) * SQ] = part[:, q * 2 * SQ : q * 2 * SQ + SQ]
        y[:, CL + q * SQ : CL + (q + 1) * SQ] = \
            part[:, q * 2 * SQ + SQ : (q + 1) * 2 * SQ]
    out = y.T + np.asarray(inputs["input_sequence"], np.float32) * np.asarray(
        inputs["D"], np.float32
    )
    return np.ascontiguousarray(out)
